# revision 13
# baseline (speedup 1.0000x reference)
"""Trainium2 Bass kernel for nn_PizzaBurningEffect.

Reference computation (per batch b):
    ew[h,w]   : fixed edge-weight grid (input-independent)
    spots     = max_s exp(-((x_w-sx)^2+(y_h-sy)^2)/(2 r_s^2)) * sint_s
    bm        = clip(max(ew, spots) * burn_b, 0, 1)
    out[c]    = img[c] * (1 - kappa_c * burn_b * max(ew, spots)),
                kappa_c = 1 - dark_c
(The clips are no-ops: every operand is in [0,1) and bm <= 0.8.)

Device strategy (p-norm max on the tensor engine):
    max_s g_s ~= (sum_s g_s^32)^(1/32)
The 32nd powers are separable: g_s^32 = gyp_s(h) * gxp_s(w), with the tiny
1-D tables gyp/gxp computed on the host (scaled by sqrt(LAM)=3.16e18 each so
fp32/bf16 dynamic range covers g in [0.017, 1]; smaller factors flush to 0).
Per 128-row chunk the sum over s is ONE 8x128 x 8x512 bf16 matmul into PSUM.
The 1/32 root is a single ACT Exp on the *bitcast-int32* view of the PSUM
sum: the int32 pattern of an fp32 is linear in log2 (max bit-log error
0.086 log2 / 32 -> <0.1% after centring), so Exp(scale*I + bias) with
scale = ln2/(32*2^23) computes (S/LAM)^(1/32) over the full fp32 range.
A small deflation delta folded into the Exp bias centres the p-norm
overshoot.

Engine split (v3): whole-batch granularity.  ACT does one Exp per batch
([P, 4*W]); DVE does the ew-max, F0/F1 and the blends; the otherwise-idle
GpSimd engine computes the F2 plane so the ACT stream stays short (its
serial stream was the tail gate in v2).

DMA (v3): img/out travel as bf16 in a partition-major [BL, P, K*C*W]
layout (one whole-batch dma_start = 128 contiguous 12 KB descriptors).
ALL img loads AND out stores ride the SP HW-DGE ring: one ring drains via
all 16 SDMA engines at full HBM rate, and with loads enqueued up-front and
stores appended as blends complete the ring never idles from first to
last byte.  Tables+ew ride the ACT ring at the start.  The last batch is
stored in small pieces so the final store's HBM write receipt is short.
~14 dma_starts total: the NEFF-exit sem-drain epilogue (~8 us, fixed) and
the shared-DGE issue serialization stay off the critical path.

Sharding: pure data parallel, 4 batches per core on 8 cores.
"""

import numpy as np
import ml_dtypes

import concourse.bacc as bacc
import concourse.bass as bass
from concourse import mybir
from concourse.tile import TileContext
from concourse.bass_utils import run_bass_kernel_spmd

B, C, H, W, S = 32, 3, 512, 512, 8
NCORES = 8
BL = B // NCORES          # batches per core
P = 128                   # partitions
K = H // P                # row chunks per image
SR = S                    # matmul contraction rows
KP = K * P                # flattened gyp width per batch (512)
F1 = C * W                # free elems per chunk (1536)
DT = mybir.dt.float32
DTH = mybir.dt.float16    # mask chain
DTB = mybir.dt.bfloat16   # img/out + power tables
NPB = ml_dtypes.bfloat16

BURN_MIN, BURN_MAX = 0.2, 0.8
DARK = np.array([0.7, 0.4, 0.3], dtype=np.float64)

PNORM = 32.0
LAM = 1e37                # sum scale; sqrt(LAM) per 1-D factor
DELTA = 0.0065            # deflation centring the p-norm overshoot
SIGMA = -0.0430           # bit-log centring constant
EXP_SCALE = float(np.log(2.0) / (PNORM * 2.0 ** 23))
EXP_BIAS = float(np.log(2.0) * (-127.0 - SIGMA) / PNORM
                 - np.log(LAM) / PNORM + np.log1p(-DELTA))

F2_ON_GPSIMD = False       # A/B knob: F2 plane on GpSimd vs ACT


def _build_program():
    nc = bacc.Bacc("TRN2", target_bir_lowering=False, debug=False,
                   num_devices=NCORES)

    img = nc.dram_tensor("img", [BL, P, K * F1], DTB, kind="ExternalInput")
    tabs = nc.dram_tensor("tabs", [SR, BL, KP + W], DTB, kind="ExternalInput")
    ew = nc.dram_tensor("ew", [P, K * W + 2 * BL * C], DTH,
                        kind="ExternalInput")
    out = nc.dram_tensor("out", [BL, P, K * F1], DTB, kind="ExternalOutput")

    mx = mybir.AluOpType.max
    mult = mybir.AluOpType.mult
    add = mybir.AluOpType.add

    with TileContext(nc) as tc:
        with (
            tc.tile_pool(name="singles", bufs=1) as singles,
            tc.tile_pool(name="imgp", bufs=1) as imgp,
            tc.tile_pool(name="outp", bufs=1) as outp,
            tc.tile_pool(name="spp", bufs=2) as spp,
            tc.tile_pool(name="bmp", bufs=2) as bmp,
            tc.tile_pool(name="fp", bufs=2) as fpool,
            tc.psum_pool(name="qp", bufs=2) as qpool,
        ):
            # Everything rides the SP ring, small gating loads first:
            # tabs/s1/ew (0.58 MB) complete early so PE/DVE can start,
            # then the four batch image loads stream back-to-back.
            # (The other HW-DGE ring starves when this one is busy, so
            # putting the small loads there would delay them by ~10 us.)
            tabs_t = singles.tile([SR, BL, KP + W], DTB)
            nc.sync.dma_start(out=tabs_t[:], in_=tabs[:])
            ew_full = singles.tile([P, K * W + 2 * BL * C], DTH)
            nc.sync.dma_start(out=ew_full[:], in_=ew[:])
            ew_t = ew_full[:, :K * W].rearrange("p (k w) -> p k w", k=K)
            s1_t = ew_full[:, K * W:].bitcast(DT).rearrange(
                "p (b c) -> p b c", b=BL)

            img_ts = []
            for b in range(BL):
                img_t = imgp.tile([P, K, F1], DTB, name=f"img_t{b}")
                if b == 0:
                    nc.sync.dma_start(out=img_t[:, 0:2, :],
                                      in_=img[b, :, 0:2 * F1])
                    nc.sync.dma_start(out=img_t[:, 2:4, :],
                                      in_=img[b, :, 2 * F1:4 * F1])
                else:
                    nc.sync.dma_start(out=img_t[:], in_=img[b])
                img_ts.append(img_t)

            bias_t = singles.tile([P, 1], DT)
            nc.vector.memset(bias_t[:], EXP_BIAS)
            warm = singles.tile([P, 2], DT)
            nc.vector.memset(warm[:, 0:1], 1.0)
            nc.scalar.activation(out=warm[:, 1:2], in_=warm[:, 0:1],
                                 func=mybir.ActivationFunctionType.Exp)

            out_ts = [None] * BL

            def issue_store(b, lo, hi):
                nc.sync.dma_start(
                    out=out[b, :, lo * F1:hi * F1],
                    in_=out_ts[b][:, lo:hi, :].rearrange("p k f -> p (k f)"))

            for b in range(BL):
                out_ts[b] = outp.tile([P, K, F1], DTB, name=f"out_t{b}")

                # mask chain; batch 0 in half-batch pieces so DVE starts
                # as soon as the first Exp half is done
                q = qpool.tile([P, K, W], DT)
                for k in range(K):
                    nc.tensor.matmul(
                        q[:, k, :], tabs_t[:, b, k * P:(k + 1) * P],
                        tabs_t[:, b, KP:KP + W])
                sp = spp.tile([P, K, W], DTH)
                bm = bmp.tile([P, K, W], DTH)
                f_t = fpool.tile([P, K, C, W], DTH)
                pieces = ([(0, 1), (1, 2), (2, 4)] if b == 0 else
                          [(0, 4)])
                dve_cs = (0, 1)
                act_cs = (2,)
                for lo, hi in pieces:
                    nc.scalar.activation(
                        out=sp[:, lo:hi, :],
                        in_=q[:, lo:hi, :].bitcast(mybir.dt.int32),
                        func=mybir.ActivationFunctionType.Exp,
                        bias=bias_t[:], scale=EXP_SCALE)
                    nc.vector.tensor_tensor(
                        out=bm[:, lo:hi, :], in0=sp[:, lo:hi, :],
                        in1=ew_t[:, lo:hi, :], op=mx)
                    for c in dve_cs:
                        nc.vector.tensor_scalar(
                            out=f_t[:, lo:hi, c, :], in0=bm[:, lo:hi, :],
                            scalar1=s1_t[:, b, c:c + 1], scalar2=1.0,
                            op0=mult, op1=add)
                    for c in act_cs:
                        nc.scalar.activation(
                            out=f_t[:, lo:hi, c, :], in_=bm[:, lo:hi, :],
                            func=mybir.ActivationFunctionType.Identity,
                            bias=1.0, scale=s1_t[:, b, c:c + 1])

                # blends + stores (all stores on the idle SP ring)
                def blend(lo, hi):
                    nc.vector.tensor_tensor(
                        out=out_ts[b][:, lo:hi, :],
                        in0=img_ts[b][:, lo:hi, :],
                        in1=f_t[:, lo:hi, :, :].rearrange(
                            "p k c w -> p k (c w)"),
                        op=mult)

                if b == 0:
                    blend(0, 2)
                    issue_store(b, 0, 2)
                    blend(2, 4)
                    issue_store(b, 2, 4)
                elif b == 1:
                    blend(0, 4)
                    issue_store(b, 0, 4)
                elif b == 2:
                    # split so the ring gets the first half ~2 us earlier
                    blend(0, 2)
                    issue_store(b, 0, 2)
                    blend(2, 4)
                    issue_store(b, 2, 4)
                else:
                    # last batch: small pieces so the tail drains fast
                    blend(0, 2)
                    issue_store(b, 0, 2)
                    blend(2, 3)
                    issue_store(b, 2, 3)
                    blend(3, 4)
                    issue_store(b, 3, 4)

    nc.compile()
    return nc


_NC = None


def _get_nc():
    global _NC
    if _NC is None:
        _NC = _build_program()
    return _NC


def _host_tables(u_xy, u_radius, u_intensity, u_burn):
    """1-D 32nd-power tables (float64 host math, bf16 on device)."""
    u_xy = np.asarray(u_xy, np.float64)
    u_radius = np.asarray(u_radius, np.float64)
    u_intensity = np.asarray(u_intensity, np.float64)
    u_burn = np.asarray(u_burn, np.float64)

    y = np.linspace(-1.0, 1.0, H)
    x = np.linspace(-1.0, 1.0, W)

    spot_xy = 2.0 * u_xy - 1.0
    sx = spot_xy[..., 0]                   # [B,S]
    sy = spot_xy[..., 1]
    radius = 0.05 + 0.15 * u_radius
    sint = 0.5 + 0.5 * u_intensity
    inv2r2 = 1.0 / (2.0 * radius ** 2)
    burn = BURN_MIN + (BURN_MAX - BURN_MIN) * u_burn   # [B]

    lamh_log = 0.5 * np.log(LAM)
    # log of (sint*gx)^32 * sqrt(LAM) and gy^32 * sqrt(LAM)
    tx = PNORM * (-((x[None, None, :] - sx[..., None]) ** 2)
                  * inv2r2[..., None] + np.log(sint)[..., None]) + lamh_log
    ty = PNORM * (-((y[None, None, :] - sy[..., None]) ** 2)
                  * inv2r2[..., None]) + lamh_log
    gxp = np.where(tx > -87.0, np.exp(tx), 0.0)        # [B,S,W]
    gyp = np.where(ty > -87.0, np.exp(ty), 0.0)        # [B,S,H]

    # device layouts: tabs[s, b, :KP] = gyp (k-major), tabs[s, b, KP:] = gxp
    gyp_lay = gyp.reshape(B, SR, KP).transpose(1, 0, 2)   # [SR,B,KP]
    gxp_lay = gxp.transpose(1, 0, 2)                      # [SR,B,W]
    tabs_lay = np.ascontiguousarray(
        np.concatenate([gyp_lay, gxp_lay], axis=2)).astype(NPB)

    kappa = 1.0 - DARK                                 # [C]
    s1 = -(burn[:, None] * kappa[None, :])             # [B,C]
    s1_lay = np.ascontiguousarray(np.broadcast_to(
        s1.astype(np.float32), (P, B, C)))
    return tabs_lay, s1_lay


def _edge_weight():
    y = np.linspace(-1.0, 1.0, H)
    x = np.linspace(-1.0, 1.0, W)
    yc, xc = np.meshgrid(y, x, indexing="ij")
    dist = np.sqrt(xc ** 2 + yc ** 2)
    ew = np.exp(2.0 * (dist - 0.7))
    ew = (ew - ew.min()) / (ew.max() - ew.min() + 1e-6)
    # ew_lay[p, k*W+w] = ew[k*P+p, w]
    return np.ascontiguousarray(
        ew.reshape(K, P, W).transpose(1, 0, 2).astype(np.float16)
    ).reshape(P, K * W)


_EW = None


def kernel(img, u_xy, u_radius, u_intensity, u_burn, _run_kwargs=None):
    global _EW
    img = np.asarray(img, np.float32)
    # pack to [B, P, K*C*W] bf16: one fully-contiguous 12KB descriptor per
    # partition per batch
    img_dev = np.ascontiguousarray(
        img.reshape(B, C, K, P, W).transpose(0, 3, 2, 1, 4)
    ).astype(NPB).reshape(B, P, K * F1)

    tabs_lay, s1_lay = _host_tables(u_xy, u_radius, u_intensity, u_burn)
    if _EW is None:
        _EW = _edge_weight()

    nc = _get_nc()
    core_ids = list(range(NCORES))
    in_maps = []
    for i in core_ids:
        lo, hi = i * BL, (i + 1) * BL
        s1_core = np.ascontiguousarray(s1_lay[:, lo:hi]).view(np.uint16)
        ew_core = np.ascontiguousarray(
            np.concatenate([_EW.view(np.uint16),
                            s1_core.reshape(P, 2 * BL * C)], axis=1)
        ).view(np.float16)
        in_maps.append({
            "img": img_dev[lo:hi],
            "tabs": np.ascontiguousarray(tabs_lay[:, lo:hi]),
            "ew": ew_core,
        })
    res = run_bass_kernel_spmd(nc, in_maps, core_ids, **(_run_kwargs or {}))
    out_dev = np.concatenate(
        [np.asarray(res.results[i]["out"]) for i in core_ids], axis=0)
    out = np.ascontiguousarray(
        out_dev.reshape(B, P, K, C, W).transpose(0, 3, 2, 1, 4)
    ).astype(np.float32).reshape(B, C, H, W)
    if _run_kwargs:
        kernel._last_results = res
    return out


# revision 14
# speedup vs baseline: 1.0405x; 1.0405x over previous
"""Trainium2 Bass kernel for nn_PizzaBurningEffect.

Reference computation (per batch b):
    ew[h,w]   : fixed edge-weight grid (input-independent)
    spots     = max_s exp(-((x_w-sx)^2+(y_h-sy)^2)/(2 r_s^2)) * sint_s
    bm        = clip(max(ew, spots) * burn_b, 0, 1)
    out[c]    = img[c] * (1 - kappa_c * burn_b * max(ew, spots)),
                kappa_c = 1 - dark_c
(The clips are no-ops: every operand is in [0,1) and bm <= 0.8.)

Device strategy (p-norm max on the tensor engine):
    max_s g_s ~= (sum_s g_s^32)^(1/32)
The 32nd powers are separable: g_s^32 = gyp_s(h) * gxp_s(w), with the tiny
1-D tables gyp/gxp computed on the host (scaled by sqrt(LAM)=3.16e18 each so
fp32/bf16 dynamic range covers g in [0.017, 1]; smaller factors flush to 0).
Per 128-row chunk the sum over s is ONE 8x128 x 8x512 bf16 matmul into PSUM.
The 1/32 root is a single ACT Exp on the *bitcast-int32* view of the PSUM
sum: the int32 pattern of an fp32 is linear in log2 (max bit-log error
0.086 log2 / 32 -> <0.1% after centring), so Exp(scale*I + bias) with
scale = ln2/(32*2^23) computes (S/LAM)^(1/32) over the full fp32 range.
A small deflation delta folded into the Exp bias centres the p-norm
overshoot.

Engine split (v3): whole-batch granularity.  ACT does one Exp per batch
([P, 4*W]); DVE does the ew-max, F0/F1 and the blends; the otherwise-idle
GpSimd engine computes the F2 plane so the ACT stream stays short (its
serial stream was the tail gate in v2).

DMA (v3): img/out travel as bf16 in a partition-major [BL, P, K*C*W]
layout (one whole-batch dma_start = 128 contiguous 12 KB descriptors).
ALL img loads AND out stores ride the SP HW-DGE ring: one ring drains via
all 16 SDMA engines at full HBM rate, and with loads enqueued up-front and
stores appended as blends complete the ring never idles from first to
last byte.  Tables+ew ride the ACT ring at the start.  The last batch is
stored in small pieces so the final store's HBM write receipt is short.
~14 dma_starts total: the NEFF-exit sem-drain epilogue (~8 us, fixed) and
the shared-DGE issue serialization stay off the critical path.

Sharding: pure data parallel, 4 batches per core on 8 cores.
"""

import numpy as np
import ml_dtypes

import concourse.bacc as bacc
import concourse.bass as bass
from concourse import mybir
from concourse.tile import TileContext
from concourse.bass_utils import run_bass_kernel_spmd

B, C, H, W, S = 32, 3, 512, 512, 8
NCORES = 8
BL = B // NCORES          # batches per core
P = 128                   # partitions
K = H // P                # row chunks per image
SR = S                    # matmul contraction rows
KP = K * P                # flattened gyp width per batch (512)
F1 = C * W                # free elems per chunk (1536)
DT = mybir.dt.float32
DTH = mybir.dt.float16    # mask chain
DTB = mybir.dt.bfloat16   # img/out + power tables
NPB = ml_dtypes.bfloat16

BURN_MIN, BURN_MAX = 0.2, 0.8
DARK = np.array([0.7, 0.4, 0.3], dtype=np.float64)

PNORM = 32.0
LAM = 1e37                # sum scale; sqrt(LAM) per 1-D factor
DELTA = 0.0065            # deflation centring the p-norm overshoot
SIGMA = -0.0430           # bit-log centring constant
EXP_SCALE = float(np.log(2.0) / (PNORM * 2.0 ** 23))
EXP_BIAS = float(np.log(2.0) * (-127.0 - SIGMA) / PNORM
                 - np.log(LAM) / PNORM + np.log1p(-DELTA))

F2_ON_GPSIMD = False       # A/B knob: F2 plane on GpSimd vs ACT


def _build_program():
    nc = bacc.Bacc("TRN2", target_bir_lowering=False, debug=False,
                   num_devices=NCORES)

    img = nc.dram_tensor("img", [BL, P, K * F1], DTB, kind="ExternalInput")
    tabs = nc.dram_tensor("tabs", [SR, BL, KP + W], DTB, kind="ExternalInput")
    ew = nc.dram_tensor("ew", [P, K * W + 2 * BL * C], DTH,
                        kind="ExternalInput")
    out = nc.dram_tensor("out", [BL, P, K * F1], DTB, kind="ExternalOutput")

    mx = mybir.AluOpType.max
    mult = mybir.AluOpType.mult
    add = mybir.AluOpType.add

    with TileContext(nc) as tc:
        with (
            tc.tile_pool(name="singles", bufs=1) as singles,
            tc.tile_pool(name="imgp", bufs=1) as imgp,
            tc.tile_pool(name="outp", bufs=1) as outp,
            tc.tile_pool(name="spp", bufs=3) as spp,
            tc.tile_pool(name="bmp", bufs=3) as bmp,
            tc.tile_pool(name="fp", bufs=3) as fpool,
            tc.psum_pool(name="qp", bufs=2) as qpool,
        ):
            # Everything rides the SP ring, small gating loads first:
            # tabs/s1/ew (0.58 MB) complete early so PE/DVE can start,
            # then the four batch image loads stream back-to-back.
            # (The other HW-DGE ring starves when this one is busy, so
            # putting the small loads there would delay them by ~10 us.)
            tabs_t = singles.tile([SR, BL, KP + W], DTB)
            nc.sync.dma_start(out=tabs_t[:], in_=tabs[:])
            ew_full = singles.tile([P, K * W + 2 * BL * C], DTH)
            nc.sync.dma_start(out=ew_full[:], in_=ew[:])
            ew_t = ew_full[:, :K * W].rearrange("p (k w) -> p k w", k=K)
            s1_t = ew_full[:, K * W:].bitcast(DT).rearrange(
                "p (b c) -> p b c", b=BL)

            img_ts = []
            for b in range(BL):
                img_t = imgp.tile([P, K, F1], DTB, name=f"img_t{b}")
                if b == 0:
                    nc.sync.dma_start(out=img_t[:, 0:2, :],
                                      in_=img[b, :, 0:2 * F1])
                    nc.sync.dma_start(out=img_t[:, 2:4, :],
                                      in_=img[b, :, 2 * F1:4 * F1])
                else:
                    nc.sync.dma_start(out=img_t[:], in_=img[b])
                img_ts.append(img_t)

            bias_t = singles.tile([P, 1], DT)
            nc.vector.memset(bias_t[:], EXP_BIAS)
            warm = singles.tile([P, 2], DT)
            nc.vector.memset(warm[:, 0:1], 1.0)
            nc.scalar.activation(out=warm[:, 1:2], in_=warm[:, 0:1],
                                 func=mybir.ActivationFunctionType.Exp)

            out_ts = [None] * BL

            def issue_store(b, lo, hi):
                nc.sync.dma_start(
                    out=out[b, :, lo * F1:hi * F1],
                    in_=out_ts[b][:, lo:hi, :].rearrange("p k f -> p (k f)"))

            for b in range(BL):
                out_ts[b] = outp.tile([P, K, F1], DTB, name=f"out_t{b}")

                # mask chain; batch 0 in half-batch pieces so DVE starts
                # as soon as the first Exp half is done
                q = qpool.tile([P, K, W], DT)
                for k in range(K):
                    nc.tensor.matmul(
                        q[:, k, :], tabs_t[:, b, k * P:(k + 1) * P],
                        tabs_t[:, b, KP:KP + W])
                sp = spp.tile([P, K, W], DTH)
                bm = bmp.tile([P, K, W], DTH)
                f_t = fpool.tile([P, K, C, W], DTH)
                pieces = ([(0, 1), (1, 2), (2, 4)] if b == 0 else
                          [(0, 4)])
                dve_cs = (0, 1)
                act_cs = (2,)
                for lo, hi in pieces:
                    nc.scalar.activation(
                        out=sp[:, lo:hi, :],
                        in_=q[:, lo:hi, :].bitcast(mybir.dt.int32),
                        func=mybir.ActivationFunctionType.Exp,
                        bias=bias_t[:], scale=EXP_SCALE)
                    nc.vector.tensor_tensor(
                        out=bm[:, lo:hi, :], in0=sp[:, lo:hi, :],
                        in1=ew_t[:, lo:hi, :], op=mx)
                    for c in dve_cs:
                        nc.vector.tensor_scalar(
                            out=f_t[:, lo:hi, c, :], in0=bm[:, lo:hi, :],
                            scalar1=s1_t[:, b, c:c + 1], scalar2=1.0,
                            op0=mult, op1=add)
                    for c in act_cs:
                        nc.scalar.activation(
                            out=f_t[:, lo:hi, c, :], in_=bm[:, lo:hi, :],
                            func=mybir.ActivationFunctionType.Identity,
                            bias=1.0, scale=s1_t[:, b, c:c + 1])

                # blends + stores (all stores on the idle SP ring)
                def blend(lo, hi):
                    nc.vector.tensor_tensor(
                        out=out_ts[b][:, lo:hi, :],
                        in0=img_ts[b][:, lo:hi, :],
                        in1=f_t[:, lo:hi, :, :].rearrange(
                            "p k c w -> p k (c w)"),
                        op=mult)

                if b == 0:
                    blend(0, 2)
                    issue_store(b, 0, 2)
                    blend(2, 4)
                    issue_store(b, 2, 4)
                elif b == 1:
                    blend(0, 4)
                    issue_store(b, 0, 4)
                elif b == 2:
                    # split so the ring gets the first half ~2 us earlier
                    blend(0, 2)
                    issue_store(b, 0, 2)
                    blend(2, 4)
                    issue_store(b, 2, 4)
                else:
                    # last batch: small pieces so the tail drains fast
                    blend(0, 2)
                    issue_store(b, 0, 2)
                    blend(2, 3)
                    issue_store(b, 2, 3)
                    blend(3, 4)
                    issue_store(b, 3, 4)

    nc.compile()
    return nc


_NC = None


def _get_nc():
    global _NC
    if _NC is None:
        _NC = _build_program()
    return _NC


def _host_tables(u_xy, u_radius, u_intensity, u_burn):
    """1-D 32nd-power tables (float64 host math, bf16 on device)."""
    u_xy = np.asarray(u_xy, np.float64)
    u_radius = np.asarray(u_radius, np.float64)
    u_intensity = np.asarray(u_intensity, np.float64)
    u_burn = np.asarray(u_burn, np.float64)

    y = np.linspace(-1.0, 1.0, H)
    x = np.linspace(-1.0, 1.0, W)

    spot_xy = 2.0 * u_xy - 1.0
    sx = spot_xy[..., 0]                   # [B,S]
    sy = spot_xy[..., 1]
    radius = 0.05 + 0.15 * u_radius
    sint = 0.5 + 0.5 * u_intensity
    inv2r2 = 1.0 / (2.0 * radius ** 2)
    burn = BURN_MIN + (BURN_MAX - BURN_MIN) * u_burn   # [B]

    lamh_log = 0.5 * np.log(LAM)
    # log of (sint*gx)^32 * sqrt(LAM) and gy^32 * sqrt(LAM)
    tx = PNORM * (-((x[None, None, :] - sx[..., None]) ** 2)
                  * inv2r2[..., None] + np.log(sint)[..., None]) + lamh_log
    ty = PNORM * (-((y[None, None, :] - sy[..., None]) ** 2)
                  * inv2r2[..., None]) + lamh_log
    gxp = np.where(tx > -87.0, np.exp(tx), 0.0)        # [B,S,W]
    gyp = np.where(ty > -87.0, np.exp(ty), 0.0)        # [B,S,H]

    # device layouts: tabs[s, b, :KP] = gyp (k-major), tabs[s, b, KP:] = gxp
    gyp_lay = gyp.reshape(B, SR, KP).transpose(1, 0, 2)   # [SR,B,KP]
    gxp_lay = gxp.transpose(1, 0, 2)                      # [SR,B,W]
    tabs_lay = np.ascontiguousarray(
        np.concatenate([gyp_lay, gxp_lay], axis=2)).astype(NPB)

    kappa = 1.0 - DARK                                 # [C]
    s1 = -(burn[:, None] * kappa[None, :])             # [B,C]
    s1_lay = np.ascontiguousarray(np.broadcast_to(
        s1.astype(np.float32), (P, B, C)))
    return tabs_lay, s1_lay


def _edge_weight():
    y = np.linspace(-1.0, 1.0, H)
    x = np.linspace(-1.0, 1.0, W)
    yc, xc = np.meshgrid(y, x, indexing="ij")
    dist = np.sqrt(xc ** 2 + yc ** 2)
    ew = np.exp(2.0 * (dist - 0.7))
    ew = (ew - ew.min()) / (ew.max() - ew.min() + 1e-6)
    # ew_lay[p, k*W+w] = ew[k*P+p, w]
    return np.ascontiguousarray(
        ew.reshape(K, P, W).transpose(1, 0, 2).astype(np.float16)
    ).reshape(P, K * W)


_EW = None


def kernel(img, u_xy, u_radius, u_intensity, u_burn, _run_kwargs=None):
    global _EW
    img = np.asarray(img, np.float32)
    # pack to [B, P, K*C*W] bf16: one fully-contiguous 12KB descriptor per
    # partition per batch
    img_dev = np.ascontiguousarray(
        img.reshape(B, C, K, P, W).transpose(0, 3, 2, 1, 4)
    ).astype(NPB).reshape(B, P, K * F1)

    tabs_lay, s1_lay = _host_tables(u_xy, u_radius, u_intensity, u_burn)
    if _EW is None:
        _EW = _edge_weight()

    nc = _get_nc()
    core_ids = list(range(NCORES))
    in_maps = []
    for i in core_ids:
        lo, hi = i * BL, (i + 1) * BL
        s1_core = np.ascontiguousarray(s1_lay[:, lo:hi]).view(np.uint16)
        ew_core = np.ascontiguousarray(
            np.concatenate([_EW.view(np.uint16),
                            s1_core.reshape(P, 2 * BL * C)], axis=1)
        ).view(np.float16)
        in_maps.append({
            "img": img_dev[lo:hi],
            "tabs": np.ascontiguousarray(tabs_lay[:, lo:hi]),
            "ew": ew_core,
        })
    res = run_bass_kernel_spmd(nc, in_maps, core_ids, **(_run_kwargs or {}))
    out_dev = np.concatenate(
        [np.asarray(res.results[i]["out"]) for i in core_ids], axis=0)
    out = np.ascontiguousarray(
        out_dev.reshape(B, P, K, C, W).transpose(0, 3, 2, 1, 4)
    ).astype(np.float32).reshape(B, C, H, W)
    if _run_kwargs:
        kernel._last_results = res
    return out


# revision 15
# speedup vs baseline: 1.1355x; 1.0913x over previous
"""Trainium2 Bass kernel for nn_PizzaBurningEffect.

Reference computation (per batch b):
    ew[h,w]   : fixed edge-weight grid (input-independent)
    spots     = max_s exp(-((x_w-sx)^2+(y_h-sy)^2)/(2 r_s^2)) * sint_s
    bm        = clip(max(ew, spots) * burn_b, 0, 1)
    out[c]    = img[c] * (1 - kappa_c * burn_b * max(ew, spots)),
                kappa_c = 1 - dark_c
(The clips are no-ops: every operand is in [0,1) and bm <= 0.8.)

Device strategy (p-norm max on the tensor engine):
    max_s g_s ~= (sum_s g_s^32)^(1/32)
The 32nd powers are separable: g_s^32 = gyp_s(h) * gxp_s(w), with the tiny
1-D tables gyp/gxp computed on the host (scaled by sqrt(LAM)=3.16e18 each so
fp32/bf16 dynamic range covers g in [0.017, 1]; smaller factors flush to 0).
Per 128-row chunk the sum over s is ONE 8x128 x 8x512 bf16 matmul into PSUM.
The 1/32 root is a single ACT Exp on the *bitcast-int32* view of the PSUM
sum: the int32 pattern of an fp32 is linear in log2 (max bit-log error
0.086 log2 / 32 -> <0.1% after centring), so Exp(scale*I + bias) with
scale = ln2/(32*2^23) computes (S/LAM)^(1/32) over the full fp32 range.
A small deflation delta folded into the Exp bias centres the p-norm
overshoot.

Engine split: whole-batch granularity (batch 0 finer so DVE starts
early).  ACT does one Exp per batch ([P, 4*W]) plus the F2 plane
(Identity with scale/bias); DVE does the ew-max, F0/F1 and the blends.
GpSimd tensor ops measured ~6x slower than DVE and poison concurrent DVE
throughput via SBUF port contention -- keep GpSimd idle.

DMA: img/out travel as bf16 in a partition-major [BL, P, K*C*W] layout
(one whole-batch dma_start = 128 contiguous 12 KB descriptors).  ALL
DMAs ride the SP HW-DGE ring in issue order: small gating loads first
(tabs, then ew with s1's bytes packed into its tail), then the four
batch image loads, then stores appended as blends complete.  One ring
drains via all 16 SDMA engines at the full ~430 GB/s HBM rate; the
OTHER ring starves whenever this one is busy (observed 64 KB taking
>5 us there), so nothing latency-critical may use it.  Stores are
whole-batch mid-run and split finer for the first/last batches so the
store stream starts early and the tail drains in small pieces.  ~13
dma_starts total keeps the shared-DGE issue serialization and the
fixed NEFF-exit semaphore-drain epilogue (~8 us) off the critical path.
Run-to-run variance of several us comes from SDMA engine 15
occasionally running slow (known TRN2 quirk) and finishing its 1/16
descriptor share alone after the other engines are done.

Sharding: pure data parallel, 4 batches per core on 8 cores.
"""

import numpy as np
import ml_dtypes

import concourse.bacc as bacc
import concourse.bass as bass
from concourse import mybir
from concourse.tile import TileContext
from concourse.bass_utils import run_bass_kernel_spmd

B, C, H, W, S = 32, 3, 512, 512, 8
NCORES = 8
BL = B // NCORES          # batches per core
P = 128                   # partitions
K = H // P                # row chunks per image
SR = S                    # matmul contraction rows
KP = K * P                # flattened gyp width per batch (512)
F1 = C * W                # free elems per chunk (1536)
DT = mybir.dt.float32
DTH = mybir.dt.float16    # mask chain
DTB = mybir.dt.bfloat16   # img/out + power tables
NPB = ml_dtypes.bfloat16

BURN_MIN, BURN_MAX = 0.2, 0.8
DARK = np.array([0.7, 0.4, 0.3], dtype=np.float64)

PNORM = 32.0
LAM = 1e37                # sum scale; sqrt(LAM) per 1-D factor
DELTA = 0.0065            # deflation centring the p-norm overshoot
SIGMA = -0.0430           # bit-log centring constant
EXP_SCALE = float(np.log(2.0) / (PNORM * 2.0 ** 23))
EXP_BIAS = float(np.log(2.0) * (-127.0 - SIGMA) / PNORM
                 - np.log(LAM) / PNORM + np.log1p(-DELTA))


def _build_program():
    nc = bacc.Bacc("TRN2", target_bir_lowering=False, debug=False,
                   num_devices=NCORES)

    img = nc.dram_tensor("img", [BL, P, K * F1], DTB, kind="ExternalInput")
    tabs = nc.dram_tensor("tabs", [SR, BL, KP + W], DTB, kind="ExternalInput")
    ew = nc.dram_tensor("ew", [P, K * W + 2 * BL * C], DTH,
                        kind="ExternalInput")
    out = nc.dram_tensor("out", [BL, P, K * F1], DTB, kind="ExternalOutput")

    mx = mybir.AluOpType.max
    mult = mybir.AluOpType.mult
    add = mybir.AluOpType.add

    with TileContext(nc) as tc:
        with (
            tc.tile_pool(name="singles", bufs=1) as singles,
            tc.tile_pool(name="imgp", bufs=1) as imgp,
            tc.tile_pool(name="outp", bufs=1) as outp,
            tc.tile_pool(name="spp", bufs=2) as spp,
            tc.tile_pool(name="bmp", bufs=2) as bmp,
            tc.tile_pool(name="fp", bufs=2) as fpool,
            tc.psum_pool(name="qp", bufs=2) as qpool,
        ):
            # Everything rides the SP ring, small gating loads first:
            # tabs/s1/ew (0.58 MB) complete early so PE/DVE can start,
            # then the four batch image loads stream back-to-back.
            # (The other HW-DGE ring starves when this one is busy, so
            # putting the small loads there would delay them by ~10 us.)
            tabs_t = singles.tile([SR, BL, KP + W], DTB)
            nc.sync.dma_start(out=tabs_t[:], in_=tabs[:])
            ew_full = singles.tile([P, K * W + 2 * BL * C], DTH)
            nc.sync.dma_start(out=ew_full[:], in_=ew[:])
            ew_t = ew_full[:, :K * W].rearrange("p (k w) -> p k w", k=K)
            s1_t = ew_full[:, K * W:].bitcast(DT).rearrange(
                "p (b c) -> p b c", b=BL)

            img_ts = []
            for b in range(BL):
                img_t = imgp.tile([P, K, F1], DTB, name=f"img_t{b}")
                if b == 0:
                    nc.sync.dma_start(out=img_t[:, 0:2, :],
                                      in_=img[b, :, 0:2 * F1])
                    nc.sync.dma_start(out=img_t[:, 2:4, :],
                                      in_=img[b, :, 2 * F1:4 * F1])
                else:
                    nc.sync.dma_start(out=img_t[:], in_=img[b])
                img_ts.append(img_t)

            bias_t = singles.tile([P, 1], DT)
            nc.vector.memset(bias_t[:], EXP_BIAS)
            warm = singles.tile([P, 2], DT)
            nc.vector.memset(warm[:, 0:1], 1.0)
            nc.scalar.activation(out=warm[:, 1:2], in_=warm[:, 0:1],
                                 func=mybir.ActivationFunctionType.Exp)

            out_ts = [None] * BL

            def issue_store(b, lo, hi):
                nc.sync.dma_start(
                    out=out[b, :, lo * F1:hi * F1],
                    in_=out_ts[b][:, lo:hi, :].rearrange("p k f -> p (k f)"))

            for b in range(BL):
                out_ts[b] = outp.tile([P, K, F1], DTB, name=f"out_t{b}")

                # mask chain; batch 0 in half-batch pieces so DVE starts
                # as soon as the first Exp half is done
                q = qpool.tile([P, K, W], DT)
                for k in range(K):
                    nc.tensor.matmul(
                        q[:, k, :], tabs_t[:, b, k * P:(k + 1) * P],
                        tabs_t[:, b, KP:KP + W])
                sp = spp.tile([P, K, W], DTH)
                bm = bmp.tile([P, K, W], DTH)
                f_t = fpool.tile([P, K, C, W], DTH)
                pieces = ([(0, 1), (1, 2), (2, 4)] if b == 0 else
                          [(0, 4)])
                dve_cs = (0, 1)
                act_cs = (2,)
                for lo, hi in pieces:
                    nc.scalar.activation(
                        out=sp[:, lo:hi, :],
                        in_=q[:, lo:hi, :].bitcast(mybir.dt.int32),
                        func=mybir.ActivationFunctionType.Exp,
                        bias=bias_t[:], scale=EXP_SCALE)
                    nc.vector.tensor_tensor(
                        out=bm[:, lo:hi, :], in0=sp[:, lo:hi, :],
                        in1=ew_t[:, lo:hi, :], op=mx)
                    for c in dve_cs:
                        nc.vector.tensor_scalar(
                            out=f_t[:, lo:hi, c, :], in0=bm[:, lo:hi, :],
                            scalar1=s1_t[:, b, c:c + 1], scalar2=1.0,
                            op0=mult, op1=add)
                    for c in act_cs:
                        nc.scalar.activation(
                            out=f_t[:, lo:hi, c, :], in_=bm[:, lo:hi, :],
                            func=mybir.ActivationFunctionType.Identity,
                            bias=1.0, scale=s1_t[:, b, c:c + 1])

                # blends + stores (all stores on the idle SP ring)
                def blend(lo, hi):
                    nc.vector.tensor_tensor(
                        out=out_ts[b][:, lo:hi, :],
                        in0=img_ts[b][:, lo:hi, :],
                        in1=f_t[:, lo:hi, :, :].rearrange(
                            "p k c w -> p k (c w)"),
                        op=mult)

                if b == 0:
                    blend(0, 2)
                    issue_store(b, 0, 2)
                    blend(2, 4)
                    issue_store(b, 2, 4)
                elif b == 1:
                    blend(0, 4)
                    issue_store(b, 0, 4)
                elif b == 2:
                    # split so the ring gets the first half ~2 us earlier
                    blend(0, 2)
                    issue_store(b, 0, 2)
                    blend(2, 4)
                    issue_store(b, 2, 4)
                else:
                    # last batch: small pieces so the tail drains fast
                    blend(0, 2)
                    issue_store(b, 0, 2)
                    blend(2, 3)
                    issue_store(b, 2, 3)
                    blend(3, 4)
                    issue_store(b, 3, 4)

    nc.compile()
    return nc


_NC = None


def _get_nc():
    global _NC
    if _NC is None:
        _NC = _build_program()
    return _NC


def _host_tables(u_xy, u_radius, u_intensity, u_burn):
    """1-D 32nd-power tables (float64 host math, bf16 on device)."""
    u_xy = np.asarray(u_xy, np.float64)
    u_radius = np.asarray(u_radius, np.float64)
    u_intensity = np.asarray(u_intensity, np.float64)
    u_burn = np.asarray(u_burn, np.float64)

    y = np.linspace(-1.0, 1.0, H)
    x = np.linspace(-1.0, 1.0, W)

    spot_xy = 2.0 * u_xy - 1.0
    sx = spot_xy[..., 0]                   # [B,S]
    sy = spot_xy[..., 1]
    radius = 0.05 + 0.15 * u_radius
    sint = 0.5 + 0.5 * u_intensity
    inv2r2 = 1.0 / (2.0 * radius ** 2)
    burn = BURN_MIN + (BURN_MAX - BURN_MIN) * u_burn   # [B]

    lamh_log = 0.5 * np.log(LAM)
    # log of (sint*gx)^32 * sqrt(LAM) and gy^32 * sqrt(LAM)
    tx = PNORM * (-((x[None, None, :] - sx[..., None]) ** 2)
                  * inv2r2[..., None] + np.log(sint)[..., None]) + lamh_log
    ty = PNORM * (-((y[None, None, :] - sy[..., None]) ** 2)
                  * inv2r2[..., None]) + lamh_log
    gxp = np.where(tx > -87.0, np.exp(tx), 0.0)        # [B,S,W]
    gyp = np.where(ty > -87.0, np.exp(ty), 0.0)        # [B,S,H]

    # device layouts: tabs[s, b, :KP] = gyp (k-major), tabs[s, b, KP:] = gxp
    gyp_lay = gyp.reshape(B, SR, KP).transpose(1, 0, 2)   # [SR,B,KP]
    gxp_lay = gxp.transpose(1, 0, 2)                      # [SR,B,W]
    tabs_lay = np.ascontiguousarray(
        np.concatenate([gyp_lay, gxp_lay], axis=2)).astype(NPB)

    kappa = 1.0 - DARK                                 # [C]
    s1 = -(burn[:, None] * kappa[None, :])             # [B,C]
    s1_lay = np.ascontiguousarray(np.broadcast_to(
        s1.astype(np.float32), (P, B, C)))
    return tabs_lay, s1_lay


def _edge_weight():
    y = np.linspace(-1.0, 1.0, H)
    x = np.linspace(-1.0, 1.0, W)
    yc, xc = np.meshgrid(y, x, indexing="ij")
    dist = np.sqrt(xc ** 2 + yc ** 2)
    ew = np.exp(2.0 * (dist - 0.7))
    ew = (ew - ew.min()) / (ew.max() - ew.min() + 1e-6)
    # ew_lay[p, k*W+w] = ew[k*P+p, w]
    return np.ascontiguousarray(
        ew.reshape(K, P, W).transpose(1, 0, 2).astype(np.float16)
    ).reshape(P, K * W)


_EW = None


def kernel(img, u_xy, u_radius, u_intensity, u_burn, _run_kwargs=None):
    global _EW
    img = np.asarray(img, np.float32)
    # pack to [B, P, K*C*W] bf16: one fully-contiguous 12KB descriptor per
    # partition per batch
    img_dev = np.ascontiguousarray(
        img.reshape(B, C, K, P, W).transpose(0, 3, 2, 1, 4)
    ).astype(NPB).reshape(B, P, K * F1)

    tabs_lay, s1_lay = _host_tables(u_xy, u_radius, u_intensity, u_burn)
    if _EW is None:
        _EW = _edge_weight()

    nc = _get_nc()
    core_ids = list(range(NCORES))
    in_maps = []
    for i in core_ids:
        lo, hi = i * BL, (i + 1) * BL
        s1_core = np.ascontiguousarray(s1_lay[:, lo:hi]).view(np.uint16)
        ew_core = np.ascontiguousarray(
            np.concatenate([_EW.view(np.uint16),
                            s1_core.reshape(P, 2 * BL * C)], axis=1)
        ).view(np.float16)
        in_maps.append({
            "img": img_dev[lo:hi],
            "tabs": np.ascontiguousarray(tabs_lay[:, lo:hi]),
            "ew": ew_core,
        })
    res = run_bass_kernel_spmd(nc, in_maps, core_ids, **(_run_kwargs or {}))
    out_dev = np.concatenate(
        [np.asarray(res.results[i]["out"]) for i in core_ids], axis=0)
    out = np.ascontiguousarray(
        out_dev.reshape(B, P, K, C, W).transpose(0, 3, 2, 1, 4)
    ).astype(np.float32).reshape(B, C, H, W)
    if _run_kwargs:
        kernel._last_results = res
    return out
